# revision 18
# baseline (speedup 1.0000x reference)
"""Trainium2 Bass kernel for masked-softmax attention scoring.

Reference computation (B=128, T=512, K=1024, Q=1024):
    mids  = einsum("kq,bq->bk", W, query)
    s     = tanh(einsum("btk,bk->bt", key, mids) + bias)
    attn  = softmax-like: exp(s - max) * mask / sum(exp(s - max) * mask)

The max-subtraction cancels exactly in the ratio (tanh is bounded), so the
device computes  attn = exp(tanh(.)) * mask / sum_t(exp(tanh(.)) * mask).

Sharding: data-parallel over B across 8 NeuronCores (16 batches/core).

v2 strategy (vs the DVE fp32 baseline at ~117us):
  * All large operands ship as fp16 (key 32MB->16MB, W^T 4MB->2MB per
    core), halving the HBM-DMA roofline.  Scores pass through a heavily
    saturated tanh (|score| std ~59), so fp16 rounding perturbs the final
    attention weights by rel_l2 ~1.4e-3 -- far inside the 2e-2 gate.
  * The per-batch dot products scores[b,t] = key[b,t,:].mids[b,:] run on
    the TENSOR engine (1 col/cycle at fp16 = ~27us for the 64K streamed
    columns, hidden under the ~45us key DMA) instead of the DVE, whose
    1x custom-op rate (~1.04ns/elem) would otherwise become the new
    bottleneck at ~73us.  The stationary is mids^T [128k x 16b]; each
    matmul streams a key chunk [128k x 512t]; out [16, 512] accumulates
    over the 8 k-chunks in a PSUM bank and only row b=w is kept.
  * Key is host-transposed to [p=k%128, w, kc, t] fp16 so each window's
    chunk is one contiguous-per-partition 1MB DMA; 16 chunks alternate
    across both HWDGE rings behind the W^T/query prologue.  The whole
    key fits in SBUF, so every DMA issues immediately (no recycling).
  * Per window: one Act op fuses the PSUM drain with tanh(+bias) into
    the [16,512] score tile; bulk epilogue does exp, mask-mul+row-sum
    (DVE affine_mul_reduce), reciprocal, scale, and the output DMA.
"""

import sys

if "/opt/trn_rl_repo" not in sys.path:
    sys.path.insert(0, "/opt/trn_rl_repo")

from contextlib import ExitStack

import numpy as np

# ---- problem constants (hardcoded per spec) ----
B, T, K, Q = 128, 512, 1024, 1024
NCORES = 8
BS = B // NCORES          # 16 batches per core (= windows)
P = 128                   # SBUF partitions
KC = K // P               # 8 contraction chunks for the score matmuls
QC = Q // P               # 8 contraction chunks for the mids matmul
WIN_BUFS = 6              # PSUM window-accumulator pool depth

_STATE: dict = {}


def _build_nc():
    import concourse.tile as tile
    from concourse import bacc, mybir

    f32 = mybir.dt.float32
    f16 = mybir.dt.float16
    nc = bacc.Bacc()

    qt_e = nc.declare_dram_parameter("qt", [P, QC, BS], f16, isOutput=False)
    wt_e = nc.declare_dram_parameter("wt", [P, KC, QC, P], f16, isOutput=False)
    key_e = nc.declare_dram_parameter("keyt", [P, BS, KC, T], f16, isOutput=False)
    # mask [16,512] ++ bias [16,1] ++ eye [16,16] packed into one DMA
    small_e = nc.declare_dram_parameter("small", [BS, T + 1 + BS], f32, isOutput=False)
    out_e = nc.declare_dram_parameter("out", [BS, T], f32, isOutput=True)

    with tile.TileContext(nc) as tc, ExitStack() as ctx:
        const = ctx.enter_context(tc.tile_pool(name="const", bufs=1))
        psum = ctx.enter_context(tc.tile_pool(name="psum", bufs=1, space="PSUM"))
        wpool = ctx.enter_context(
            tc.tile_pool(name="win", bufs=WIN_BUFS, space="PSUM")
        )

        # ---- prologue loads split across BOTH HWDGE rings ----
        # ring A (sync/SP): W^T half; ring B (scalar/Act): query, the
        # other W^T half, the packed small tensors.  ~1MB per ring.
        qt_sb = const.tile([P, QC, BS], f16)
        nc.scalar.dma_start(out=qt_sb[:], in_=qt_e[:])
        wt_sb = const.tile([P, KC, QC, P], f16)
        nc.sync.dma_start(out=wt_sb[:, 0:4, :, :], in_=wt_e[:, 0:4, :, :])
        nc.scalar.dma_start(out=wt_sb[:, 4:8, :, :], in_=wt_e[:, 4:8, :, :])
        small_sb = const.tile([BS, T + 1 + BS], f32)
        nc.scalar.dma_start(out=small_sb[:], in_=small_e[:])

        # ---- mids^T tiles: midsT[kt][p, b] = mids[b, kt*128+p] ----
        # mids[b, k] = sum_q W[k, q] query[b, q]; accumulate over the 8
        # q-chunks with W^T chunks stationary, then cast fp32->fp16 so the
        # tiles can serve as fp16 stationaries for the score matmuls.
        midsT_ps = psum.tile([P, KC, BS], f32)
        midsT_sb = const.tile([P, KC, BS], f16)
        for kt in range(KC):
            for qc in range(QC):
                nc.tensor.matmul(
                    midsT_ps[:, kt, :],
                    lhsT=wt_sb[:, kt, qc, :],
                    rhs=qt_sb[:, qc, :],
                    start=(qc == 0),
                    stop=(qc == QC - 1),
                )
            nc.vector.tensor_copy(midsT_sb[:, kt, :], midsT_ps[:, kt, :])

        # ---- scores: stream key windows through the PE ----
        # window w == batch b; chunk w is [128, KC, T] fp16 (1MB); even
        # chunks ride ring A (sync), odd chunks ring B (scalar).  Per
        # chunk: 8 accumulating matmuls into a PSUM bank out[b', t] =
        # sum_k mids[b', k] key[w, t, k]; only row b'=w is real.  Engines
        # cannot address single partitions (quadrant rule), so the Act
        # engine tanh's the FULL [16, T] tile out of PSUM and a DVE
        # affine_then_add with a one-hot per-partition scale (eye column
        # w) accumulates row w into a ping-pong assembly of tanh(scores +
        # bias); garbage rows are scaled by 0.
        #
        # The window drain is a single DVE affine_then_add reading the
        # PSUM tile directly (no Act involvement: Act is in-order and its
        # dma_starts block on HWDGE ring space, so any Act compute queued
        # behind them would stall until the ring drains -- which in v2
        # blocked PSUM recycling and stalled the PE for 10us).  tanh is
        # applied in bulk on the assembled scores afterwards.
        key_sb = const.tile([P, BS, KC, T], f16)
        acc_a = const.tile([BS, T], f32)
        acc_b = const.tile([BS, T], f32)
        acc = [acc_a, acc_b]
        # Chunks ride three queues: the two HWDGE rings (sync/scalar) and
        # a GpSimd software-DGE queue carrying four mid-run windows.  The
        # last two windows ship as 4 x 256KB pieces each, alternating the
        # HWDGE rings, so the final matmuls chase the DMA tail.
        GP_WINS = (2, 5, 8, 11)
        for w in range(0, BS - 2):
            if w in GP_WINS:
                nc.gpsimd.dma_start(
                    out=key_sb[:, w, :, :], in_=key_e[:, w, :, :]
                )
        for w in range(0, BS - 2):
            if w % 2 == 0 and w not in GP_WINS:
                nc.sync.dma_start(
                    out=key_sb[:, w, :, :], in_=key_e[:, w, :, :]
                )
        for w in (BS - 2, BS - 1):
            for j in (0, 2):
                nc.sync.dma_start(
                    out=key_sb[:, w, 2 * j : 2 * j + 2, :],
                    in_=key_e[:, w, 2 * j : 2 * j + 2, :],
                )
        for w in range(1, BS - 2):
            if w % 2 == 1 and w not in GP_WINS:
                nc.scalar.dma_start(
                    out=key_sb[:, w, :, :], in_=key_e[:, w, :, :]
                )
        for w in (BS - 2, BS - 1):
            for j in (1, 3):
                nc.scalar.dma_start(
                    out=key_sb[:, w, 2 * j : 2 * j + 2, :],
                    in_=key_e[:, w, 2 * j : 2 * j + 2, :],
                )
        for w in range(BS):
            win = wpool.tile([P, T], f32, tag="win")
            for kc in range(KC):
                nc.tensor.matmul(
                    win[0:BS, :],
                    lhsT=midsT_sb[:, kc, :],
                    rhs=key_sb[:, w, kc, :],
                    start=(kc == 0),
                    stop=(kc == KC - 1),
                )
            if w == 0:
                nc.vector.tensor_scalar_mul(
                    acc[0][:], win[0:BS, :], small_sb[:, T + 1 : T + 2]
                )
            else:
                nc.vector.affine_then_add(
                    out=acc[w % 2][:],
                    in0=win[0:BS, :],
                    in1=acc[(w - 1) % 2][:],
                    scale=small_sb[:, T + 1 + w : T + 2 + w],
                    bias=0.0,
                )
        scores_sb = acc[(BS - 1) % 2]

        # ---- epilogue: tanh(+bias), exp, mask, normalize ----
        tanh_sb = const.tile([BS, T], f32)
        nc.scalar.activation(
            out=tanh_sb[:],
            in_=scores_sb[:],
            func=mybir.ActivationFunctionType.Tanh,
            bias=small_sb[:, T : T + 1],
            scale=1.0,
        )
        exp_sb = const.tile([BS, T], f32)
        nc.scalar.activation(
            out=exp_sb[:], in_=tanh_sb[:], func=mybir.ActivationFunctionType.Exp
        )
        em_sb = const.tile([BS, T], f32)
        rowsum = const.tile([BS, 1], f32)
        nc.vector.affine_mul_reduce(
            out=em_sb[:],
            accum_out=rowsum[:],
            in0=exp_sb[:],
            in1=small_sb[:, 0:T],
            scale=1.0,
            bias=0.0,
        )
        rinv = const.tile([BS, 1], f32)
        nc.vector.reciprocal(out=rinv[:], in_=rowsum[:])
        attn_sb = const.tile([BS, T], f32)
        nc.vector.tensor_scalar_mul(attn_sb[:], em_sb[:], rinv[:])
        nc.sync.dma_start(out=out_e[:], in_=attn_sb[:])

    nc.compile()
    return nc


def _get_nc():
    if "nc" not in _STATE:
        _STATE["nc"] = _build_nc()
    return _STATE["nc"]


def _make_in_maps(query, key, mask, W, bias):
    query = np.asarray(query, dtype=np.float32)
    key = np.asarray(key, dtype=np.float32)
    mask = np.asarray(mask, dtype=np.float32)
    W = np.asarray(W, dtype=np.float32)
    bias = np.asarray(bias, dtype=np.float32).reshape(-1)

    # wt[p, kt, qc, k'] = W[kt*128+k', qc*128+p]
    WT = np.ascontiguousarray(
        W.reshape(KC, P, QC, P).transpose(3, 0, 2, 1).astype(np.float16)
    )
    biasb = np.broadcast_to(bias[:1][None, :], (BS, 1)).astype(np.float32)
    eye = np.eye(BS, dtype=np.float32)

    in_maps = []
    for i in range(NCORES):
        sh = slice(i * BS, (i + 1) * BS)
        # keyt[p, w, kc, t] = key[b0+w, t, kc*128+p]
        kt = np.ascontiguousarray(
            key[sh].reshape(BS, T, KC, P).transpose(3, 0, 2, 1).astype(np.float16)
        )
        # qt[p, qc, b] = query[b0+b, qc*128+p]
        qt = np.ascontiguousarray(
            query[sh].reshape(BS, QC, P).transpose(2, 1, 0).astype(np.float16)
        )
        small = np.ascontiguousarray(
            np.concatenate([mask[sh], biasb, eye], axis=1).astype(np.float32)
        )
        in_maps.append(
            {
                "qt": qt,
                "wt": WT,
                "keyt": kt,
                "small": small,
            }
        )
    return in_maps


def _run(in_maps, **kwargs):
    from concourse.bass_utils import run_bass_kernel_spmd

    return run_bass_kernel_spmd(
        _get_nc(), in_maps, core_ids=list(range(NCORES)), **kwargs
    )


def _gather(results):
    return np.concatenate(
        [np.asarray(r["out"]).reshape(BS, T) for r in results], axis=0
    )


def kernel(query, key, mask, W, bias):
    in_maps = _make_in_maps(query, key, mask, W, bias)
    res = _run(in_maps)
    return _gather(res.results)


# revision 19
# speedup vs baseline: 1.1103x; 1.1103x over previous
"""Trainium2 Bass kernel for masked-softmax attention scoring.

Reference computation (B=128, T=512, K=1024, Q=1024):
    mids  = einsum("kq,bq->bk", W, query)
    s     = tanh(einsum("btk,bk->bt", key, mids) + bias)
    attn  = softmax-like: exp(s - max) * mask / sum(exp(s - max) * mask)

The max-subtraction cancels exactly in the ratio (tanh is bounded), so the
device computes  attn = exp(tanh(.)) * mask / sum_t(exp(tanh(.)) * mask).

Sharding: data-parallel over B across 8 NeuronCores (16 batches/core).

v2 strategy (vs the DVE fp32 baseline at ~117us):
  * All large operands ship as fp16 (key 32MB->16MB, W^T 4MB->2MB per
    core), halving the HBM-DMA roofline.  Scores pass through a heavily
    saturated tanh (|score| std ~59), so fp16 rounding perturbs the final
    attention weights by rel_l2 ~1.4e-3 -- far inside the 2e-2 gate.
  * The per-batch dot products scores[b,t] = key[b,t,:].mids[b,:] run on
    the TENSOR engine (1 col/cycle at fp16 = ~27us for the 64K streamed
    columns, hidden under the ~45us key DMA) instead of the DVE, whose
    1x custom-op rate (~1.04ns/elem) would otherwise become the new
    bottleneck at ~73us.  The stationary is mids^T [128k x 16b]; each
    matmul streams a key chunk [128k x 512t]; out [16, 512] accumulates
    over the 8 k-chunks in a PSUM bank and only row b=w is kept.
  * Key is host-transposed to [p=k%128, w, kc, t] fp16 so each window's
    chunk is one contiguous-per-partition 1MB DMA; 16 chunks alternate
    across both HWDGE rings behind the W^T/query prologue.  The whole
    key fits in SBUF, so every DMA issues immediately (no recycling).
  * Per window: one Act op fuses the PSUM drain with tanh(+bias) into
    the [16,512] score tile; bulk epilogue does exp, mask-mul+row-sum
    (DVE affine_mul_reduce), reciprocal, scale, and the output DMA.
"""

import sys

if "/opt/trn_rl_repo" not in sys.path:
    sys.path.insert(0, "/opt/trn_rl_repo")

from contextlib import ExitStack

import numpy as np

# ---- problem constants (hardcoded per spec) ----
B, T, K, Q = 128, 512, 1024, 1024
NCORES = 8
BS = B // NCORES          # 16 batches per core (= windows)
P = 128                   # SBUF partitions
KC = K // P               # 8 contraction chunks for the score matmuls
QC = Q // P               # 8 contraction chunks for the mids matmul
WIN_BUFS = 6              # PSUM window-accumulator pool depth

_STATE: dict = {}


def _build_nc():
    import concourse.tile as tile
    from concourse import bacc, mybir

    f32 = mybir.dt.float32
    f16 = mybir.dt.float16
    nc = bacc.Bacc()

    qt_e = nc.declare_dram_parameter("qt", [P, QC, BS], f16, isOutput=False)
    wt_e = nc.declare_dram_parameter("wt", [P, KC, QC, P], f16, isOutput=False)
    key_e = nc.declare_dram_parameter("keyt", [P, BS, KC, T], f16, isOutput=False)
    # mask [16,512] ++ bias [16,1] ++ eye [16,16] packed into one DMA
    small_e = nc.declare_dram_parameter("small", [BS, T + 1 + BS], f32, isOutput=False)
    out_e = nc.declare_dram_parameter("out", [BS, T], f32, isOutput=True)

    with tile.TileContext(nc) as tc, ExitStack() as ctx:
        const = ctx.enter_context(tc.tile_pool(name="const", bufs=1))
        psum = ctx.enter_context(tc.tile_pool(name="psum", bufs=1, space="PSUM"))
        wpool = ctx.enter_context(
            tc.tile_pool(name="win", bufs=WIN_BUFS, space="PSUM")
        )

        # ---- prologue loads split across BOTH HWDGE rings ----
        # ring A (sync/SP): W^T half; ring B (scalar/Act): query, the
        # other W^T half, the packed small tensors.  ~1MB per ring.
        qt_sb = const.tile([P, QC, BS], f16)
        nc.scalar.dma_start(out=qt_sb[:], in_=qt_e[:])
        wt_sb = const.tile([P, KC, QC, P], f16)
        nc.sync.dma_start(out=wt_sb[:, 0:4, :, :], in_=wt_e[:, 0:4, :, :])
        nc.scalar.dma_start(out=wt_sb[:, 4:8, :, :], in_=wt_e[:, 4:8, :, :])
        small_sb = const.tile([BS, T + 1 + BS], f32)
        nc.scalar.dma_start(out=small_sb[:], in_=small_e[:])

        # ---- mids^T tiles: midsT[kt][p, b] = mids[b, kt*128+p] ----
        # mids[b, k] = sum_q W[k, q] query[b, q]; accumulate over the 8
        # q-chunks with W^T chunks stationary, then cast fp32->fp16 so the
        # tiles can serve as fp16 stationaries for the score matmuls.
        midsT_ps = psum.tile([P, KC, BS], f32)
        midsT_sb = const.tile([P, KC, BS], f16)
        for kt in range(KC):
            for qc in range(QC):
                nc.tensor.matmul(
                    midsT_ps[:, kt, :],
                    lhsT=wt_sb[:, kt, qc, :],
                    rhs=qt_sb[:, qc, :],
                    start=(qc == 0),
                    stop=(qc == QC - 1),
                )
            nc.vector.tensor_copy(midsT_sb[:, kt, :], midsT_ps[:, kt, :])

        # ---- scores: stream key windows through the PE ----
        # window w == batch b; chunk w is [128, KC, T] fp16 (1MB); even
        # chunks ride ring A (sync), odd chunks ring B (scalar).  Per
        # chunk: 8 accumulating matmuls into a PSUM bank out[b', t] =
        # sum_k mids[b', k] key[w, t, k]; only row b'=w is real.  Engines
        # cannot address single partitions (quadrant rule), so the Act
        # engine tanh's the FULL [16, T] tile out of PSUM and a DVE
        # affine_then_add with a one-hot per-partition scale (eye column
        # w) accumulates row w into a ping-pong assembly of tanh(scores +
        # bias); garbage rows are scaled by 0.
        #
        # The window drain is a single DVE affine_then_add reading the
        # PSUM tile directly (no Act involvement: Act is in-order and its
        # dma_starts block on HWDGE ring space, so any Act compute queued
        # behind them would stall until the ring drains -- which in v2
        # blocked PSUM recycling and stalled the PE for 10us).  tanh is
        # applied in bulk on the assembled scores afterwards.
        key_sb = const.tile([P, BS, KC, T], f16)
        acc_a = const.tile([BS, T], f32)
        acc_b = const.tile([BS, T], f32)
        acc = [acc_a, acc_b]
        # Last two windows ship as 4 x 256KB pieces each, alternating
        # rings (balance preserved), so the final matmuls chase the DMA
        # tail instead of waiting on one whole 1MB chunk.  (A third
        # queue via GpSimd SWDGE was tried and degraded ALL queues --
        # aggregate fell from ~400 to ~265 GB/s; two HWDGE rings win.)
        for w in range(0, BS - 2, 2):
            nc.sync.dma_start(out=key_sb[:, w, :, :], in_=key_e[:, w, :, :])
        for w in (BS - 2, BS - 1):
            for j in (0, 2):
                nc.sync.dma_start(
                    out=key_sb[:, w, 2 * j : 2 * j + 2, :],
                    in_=key_e[:, w, 2 * j : 2 * j + 2, :],
                )
        for w in range(1, BS - 2, 2):
            nc.scalar.dma_start(out=key_sb[:, w, :, :], in_=key_e[:, w, :, :])
        for w in (BS - 2, BS - 1):
            for j in (1, 3):
                nc.scalar.dma_start(
                    out=key_sb[:, w, 2 * j : 2 * j + 2, :],
                    in_=key_e[:, w, 2 * j : 2 * j + 2, :],
                )
        for w in range(BS):
            win = wpool.tile([P, T], f32, tag="win")
            for kc in range(KC):
                nc.tensor.matmul(
                    win[0:BS, :],
                    lhsT=midsT_sb[:, kc, :],
                    rhs=key_sb[:, w, kc, :],
                    start=(kc == 0),
                    stop=(kc == KC - 1),
                )
            if w == 0:
                nc.vector.tensor_scalar_mul(
                    acc[0][:], win[0:BS, :], small_sb[:, T + 1 : T + 2]
                )
            else:
                nc.vector.affine_then_add(
                    out=acc[w % 2][:],
                    in0=win[0:BS, :],
                    in1=acc[(w - 1) % 2][:],
                    scale=small_sb[:, T + 1 + w : T + 2 + w],
                    bias=0.0,
                )
        scores_sb = acc[(BS - 1) % 2]

        # ---- epilogue: tanh(+bias), exp, mask, normalize ----
        tanh_sb = const.tile([BS, T], f32)
        nc.scalar.activation(
            out=tanh_sb[:],
            in_=scores_sb[:],
            func=mybir.ActivationFunctionType.Tanh,
            bias=small_sb[:, T : T + 1],
            scale=1.0,
        )
        exp_sb = const.tile([BS, T], f32)
        nc.scalar.activation(
            out=exp_sb[:], in_=tanh_sb[:], func=mybir.ActivationFunctionType.Exp
        )
        em_sb = const.tile([BS, T], f32)
        rowsum = const.tile([BS, 1], f32)
        nc.vector.affine_mul_reduce(
            out=em_sb[:],
            accum_out=rowsum[:],
            in0=exp_sb[:],
            in1=small_sb[:, 0:T],
            scale=1.0,
            bias=0.0,
        )
        rinv = const.tile([BS, 1], f32)
        nc.vector.reciprocal(out=rinv[:], in_=rowsum[:])
        attn_sb = const.tile([BS, T], f32)
        nc.vector.tensor_scalar_mul(attn_sb[:], em_sb[:], rinv[:])
        nc.sync.dma_start(out=out_e[:], in_=attn_sb[:])

    nc.compile()
    return nc


def _get_nc():
    if "nc" not in _STATE:
        _STATE["nc"] = _build_nc()
    return _STATE["nc"]


def _make_in_maps(query, key, mask, W, bias):
    query = np.asarray(query, dtype=np.float32)
    key = np.asarray(key, dtype=np.float32)
    mask = np.asarray(mask, dtype=np.float32)
    W = np.asarray(W, dtype=np.float32)
    bias = np.asarray(bias, dtype=np.float32).reshape(-1)

    # wt[p, kt, qc, k'] = W[kt*128+k', qc*128+p]
    WT = np.ascontiguousarray(
        W.reshape(KC, P, QC, P).transpose(3, 0, 2, 1).astype(np.float16)
    )
    biasb = np.broadcast_to(bias[:1][None, :], (BS, 1)).astype(np.float32)
    eye = np.eye(BS, dtype=np.float32)

    in_maps = []
    for i in range(NCORES):
        sh = slice(i * BS, (i + 1) * BS)
        # keyt[p, w, kc, t] = key[b0+w, t, kc*128+p]
        kt = np.ascontiguousarray(
            key[sh].reshape(BS, T, KC, P).transpose(3, 0, 2, 1).astype(np.float16)
        )
        # qt[p, qc, b] = query[b0+b, qc*128+p]
        qt = np.ascontiguousarray(
            query[sh].reshape(BS, QC, P).transpose(2, 1, 0).astype(np.float16)
        )
        small = np.ascontiguousarray(
            np.concatenate([mask[sh], biasb, eye], axis=1).astype(np.float32)
        )
        in_maps.append(
            {
                "qt": qt,
                "wt": WT,
                "keyt": kt,
                "small": small,
            }
        )
    return in_maps


def _run(in_maps, **kwargs):
    from concourse.bass_utils import run_bass_kernel_spmd

    return run_bass_kernel_spmd(
        _get_nc(), in_maps, core_ids=list(range(NCORES)), **kwargs
    )


def _gather(results):
    return np.concatenate(
        [np.asarray(r["out"]).reshape(BS, T) for r in results], axis=0
    )


def kernel(query, key, mask, W, bias):
    in_maps = _make_in_maps(query, key, mask, W, bias)
    res = _run(in_maps)
    return _gather(res.results)
